# revision 4
# baseline (speedup 1.0000x reference)
# nn_DirectionalConv on TRN2 (8 NeuronCores), Bass/Tile.
#
#   out[r] = deg_inv[r] * sum_{e: row[e]==r} edge_weight[e] * x[col[e]]
#   x: [100000, 32] f32, edge_index: [2, 1600000] i32 (row=dst, col=src)
#
# Strategy (destination-sharded, "row-per-partition" grid):
#  * Host sorts destination rows by degree and packs them into blocks of 128
#    rows; blocks are dealt to the 8 cores snake-wise (load balance). Block
#    position g on a core maps its 128 rows onto the 128 SBUF partitions.
#    Row (g, p) owns K_sched[g] edge slots (K_sched = group max degree, so a
#    single NEFF serves all cores).
#  * The per-core slot grid is [128, S] (column j = one edge per partition).
#    One stock indirect DMA per column gathers x[col] for 128 edges (TRN2's
#    dynamic-AP DGE consumes one offset per destination partition).
#  * DVE multiplies the gathered rows by the per-slot weight
#    (edge_weight * deg_inv[row], folded on the host) and segment-reduces
#    each position's K columns -> [128, 32], DMA'd out contiguously.
#  * No scatter, no collectives. The host unpermutes rows at the end.
import numpy as np

P = 128
F = 32
KSPAN = 64
N_CORES = 8

LAST_EXEC_TIME_NS = None


def _build_schedule(row, col, w, deg_inv, N):
    global KSPAN
    E = row.shape[0]
    deg = np.bincount(row, minlength=N).astype(np.int64)
    KSPAN = max(64, int(deg.max(initial=0)))  # 64 for the reference input
    B_total = -(-N // (P * N_CORES)) * N_CORES
    N_pad = B_total * P
    deg_pad = np.concatenate([deg, np.zeros(N_pad - N, np.int64)])
    order = np.argsort(deg_pad, kind="stable")
    rank = np.empty(N_pad, np.int64)
    rank[order] = np.arange(N_pad)

    G = B_total // N_CORES
    K_blk = deg_pad[order].reshape(B_total, P).max(axis=1)
    K_sched = K_blk.reshape(G, N_CORES).max(axis=1).astype(np.int64)
    assert K_sched.max(initial=0) <= KSPAN, "block max degree exceeds one span"
    slot_base = np.zeros(G + 1, np.int64)
    acc = 0
    for g in range(G):
        k = int(K_sched[g])
        if k > 0 and (acc % KSPAN) + k > KSPAN:
            acc = -(-acc // KSPAN) * KSPAN
        slot_base[g] = acc
        acc += k
    slot_base[G] = acc
    S_pp = acc
    S_pad = -(-max(S_pp, 1) // KSPAN) * KSPAN

    pr = rank[row]
    blk_e = pr // P
    p_e = (pr % P).astype(np.int64)
    g_e = blk_e // N_CORES
    j_e = blk_e % N_CORES
    core_e = np.where(g_e % 2 == 0, j_e, N_CORES - 1 - j_e)
    if E > 0:
        o = np.argsort(pr, kind="stable")
        pr_s = pr[o]
        first = np.r_[True, pr_s[1:] != pr_s[:-1]]
        run_start = np.maximum.accumulate(np.where(first, np.arange(E), 0))
        k_s = np.arange(E) - run_start
        k_e = np.empty(E, np.int64)
        k_e[o] = k_s
    else:
        k_e = np.zeros(0, np.int64)
    off_e = slot_base[g_e] + k_e

    col_slots = np.zeros((N_CORES, P, S_pad), np.int32)
    w_slots = np.zeros((N_CORES, P, S_pad), np.float32)
    wdi = (w.astype(np.float64) * deg_inv[row].astype(np.float64)).astype(np.float32)
    col_slots[core_e, p_e, off_e] = col
    w_slots[core_e, p_e, off_e] = wdi

    col_used = np.zeros((N_CORES, S_pad), bool)
    col_used[core_e, off_e] = True
    used_any = col_used.any(axis=0)

    return dict(order=order, K_sched=K_sched, slot_base=slot_base, S_pp=S_pp,
                S_pad=S_pad, G=G, col_slots=col_slots, w_slots=w_slots,
                used_any=used_any, N_pad=N_pad)


def _build_kernel(sched, N, gather_bufs=4):
    import concourse.bass as bass
    import concourse.bacc as bacc
    import concourse.tile as tile
    import concourse.mybir as mybir

    K_sched = sched["K_sched"]
    slot_base = sched["slot_base"]
    S_pad = sched["S_pad"]
    G = sched["G"]
    used_any = sched["used_any"]
    n_spans = S_pad // KSPAN

    nc = bacc.Bacc("TRN2", target_bir_lowering=False, debug=False,
                   num_devices=N_CORES)

    x = nc.dram_tensor("x", [N, F], mybir.dt.float32, kind="ExternalInput")
    cols = nc.dram_tensor("cols", [P, S_pad], mybir.dt.int32, kind="ExternalInput")
    ws = nc.dram_tensor("ws", [P, S_pad], mybir.dt.float32, kind="ExternalInput")
    out = nc.dram_tensor("out", [G * P, F], mybir.dt.float32, kind="ExternalOutput")

    with tile.TileContext(nc) as tc:
        with (
            tc.tile_pool(name="cp", bufs=1) as cp,
            tc.tile_pool(name="wp", bufs=1) as wp,
            tc.tile_pool(name="gp", bufs=gather_bufs) as gp,
            tc.tile_pool(name="rp", bufs=4) as rp,
        ):
            extents = []
            for s in range(n_spans):
                u = used_any[s * KSPAN:(s + 1) * KSPAN]
                ext = int(np.max(np.nonzero(u)[0]) + 1) if u.any() else 0
                assert u[:ext].all(), "span used columns not a prefix"
                extents.append(ext)

            # all offsets and weights resident up front (tiny: ~14 KB/partition)
            col_all = cp.tile([P, S_pad], mybir.dt.int32)
            nc.sync.dma_start(out=col_all[:], in_=cols[:])
            w_all = wp.tile([P, S_pad], mybir.dt.float32)
            nc.sync.dma_start(out=w_all[:], in_=ws[:])

            span_tiles = {}
            for s in range(n_spans):
                ext = extents[s]
                if ext == 0:
                    span_tiles[s] = None
                    continue
                g_t = gp.tile([P, KSPAN, F], mybir.dt.float32, tag="g")
                for j in range(ext):
                    jj = s * KSPAN + j
                    nc.gpsimd.indirect_dma_start(
                        out=g_t[:, j, :], out_offset=None, in_=x[:],
                        in_offset=bass.IndirectOffsetOnAxis(
                            ap=col_all[:, jj:jj + 1], axis=0))
                nc.vector.tensor_tensor(
                    out=g_t[:, 0:ext, :], in0=g_t[:, 0:ext, :],
                    in1=w_all[:, s * KSPAN:s * KSPAN + ext].to_broadcast(
                        [P, ext, F]),
                    op=mybir.AluOpType.mult)
                span_tiles[s] = g_t

            g = 0
            while g < G:
                k = int(K_sched[g])
                if k == 0:
                    ge = g
                    while ge < G and int(K_sched[ge]) == 0:
                        ge += 1
                    rz = rp.tile([P, F], mybir.dt.float32, tag="r")
                    nc.vector.memset(rz[:], 0.0)
                    for gg in range(g, ge):
                        nc.sync.dma_start(out=out[gg * P:(gg + 1) * P, :], in_=rz[:])
                    g = ge
                    continue
                s = int(slot_base[g]) // KSPAN
                ge = g + 1
                while (ge < G and int(K_sched[ge]) == k
                       and int(slot_base[ge]) == int(slot_base[ge - 1]) + k
                       and int(slot_base[ge]) // KSPAN == s):
                    ge += 1
                nrun = ge - g
                j0 = int(slot_base[g]) - s * KSPAN
                g_t = span_tiles[s]
                src = g_t[:, j0:j0 + nrun * k, :].rearrange(
                    "p (r k) f -> p r f k", k=k)
                r_t = rp.tile([P, nrun * F], mybir.dt.float32, tag="r")
                nc.vector.tensor_reduce(out=r_t[:], in_=src,
                                        axis=mybir.AxisListType.X,
                                        op=mybir.AluOpType.add)
                for i, gg in enumerate(range(g, ge)):
                    nc.sync.dma_start(out=out[gg * P:(gg + 1) * P, :],
                                      in_=r_t[:, i * F:(i + 1) * F])
                g = ge

    nc.compile()
    return nc


def _unshard(sched, core_outs, N):
    G = sched["G"]
    order = sched["order"]
    out = np.zeros((N, F), np.float32)
    g_idx = np.arange(G)
    for c in range(N_CORES):
        j = np.where(g_idx % 2 == 0, c, N_CORES - 1 - c)
        blk = g_idx * N_CORES + j
        ranks = (blk[:, None] * P + np.arange(P)).ravel()
        rows = order[ranks]
        mask = rows < N
        out[rows[mask]] = core_outs[c][mask]
    return out


def kernel(x, edge_index, edge_weight, deg_inv):
    global LAST_EXEC_TIME_NS
    import os
    from concourse.bass_utils import run_bass_kernel_spmd

    x = np.ascontiguousarray(np.asarray(x, dtype=np.float32))
    edge_index = np.asarray(edge_index, dtype=np.int32)
    edge_weight = np.asarray(edge_weight, dtype=np.float32)
    deg_inv = np.asarray(deg_inv, dtype=np.float32)
    N = x.shape[0]

    sched = _build_schedule(edge_index[0], edge_index[1], edge_weight, deg_inv, N)
    nc = _build_kernel(sched, N)
    in_maps = [{"x": x, "cols": sched["col_slots"][c], "ws": sched["w_slots"][c]}
               for c in range(N_CORES)]

    trace = bool(int(os.environ.get("KERNEL_TRACE", "0")))
    res = run_bass_kernel_spmd(nc, in_maps, core_ids=list(range(N_CORES)),
                               trace=trace)
    if trace:
        LAST_EXEC_TIME_NS = res.exec_time_ns
    return _unshard(sched, [r["out"] for r in res.results], N)


# revision 5
# speedup vs baseline: 1.0030x; 1.0030x over previous
# nn_DirectionalConv on TRN2 (8 NeuronCores), Bass/Tile.
#
#   out[r] = deg_inv[r] * sum_{e: row[e]==r} edge_weight[e] * x[col[e]]
#   x: [100000, 32] f32, edge_index: [2, 1600000] i32 (row=dst, col=src)
#
# Strategy (destination-sharded, "row-per-partition" grid):
#  * Host sorts destination rows by degree and packs them into blocks of 128
#    rows; blocks are dealt to the 8 cores snake-wise (load balance). Block
#    position g on a core maps its 128 rows onto the 128 SBUF partitions.
#    Row (g, p) owns K_sched[g] edge slots (K_sched = group max degree, so a
#    single NEFF serves all cores).
#  * The per-core slot grid is [128, S] (column j = one edge per partition).
#    One stock indirect DMA per column gathers x[col] for 128 edges (TRN2's
#    dynamic-AP DGE consumes one offset per destination partition).
#  * DVE multiplies the gathered rows by the per-slot weight
#    (edge_weight * deg_inv[row], folded on the host) and segment-reduces
#    each position's K columns -> [128, 32], DMA'd out contiguously.
#  * No scatter, no collectives. The host unpermutes rows at the end.
import numpy as np

P = 128
F = 32
KSPAN = 64
N_CORES = 8

LAST_EXEC_TIME_NS = None


def _build_schedule(row, col, w, deg_inv, N):
    global KSPAN
    E = row.shape[0]
    deg = np.bincount(row, minlength=N).astype(np.int64)
    KSPAN = max(64, int(deg.max(initial=0)))  # 64 for the reference input
    B_total = -(-N // (P * N_CORES)) * N_CORES
    N_pad = B_total * P
    deg_pad = np.concatenate([deg, np.zeros(N_pad - N, np.int64)])
    order = np.argsort(deg_pad, kind="stable")
    rank = np.empty(N_pad, np.int64)
    rank[order] = np.arange(N_pad)

    G = B_total // N_CORES
    K_blk = deg_pad[order].reshape(B_total, P).max(axis=1)
    K_sched = K_blk.reshape(G, N_CORES).max(axis=1).astype(np.int64)
    assert K_sched.max(initial=0) <= KSPAN, "block max degree exceeds one span"
    slot_base = np.zeros(G + 1, np.int64)
    acc = 0
    for g in range(G):
        k = int(K_sched[g])
        if k > 0 and (acc % KSPAN) + k > KSPAN:
            acc = -(-acc // KSPAN) * KSPAN
        slot_base[g] = acc
        acc += k
    slot_base[G] = acc
    S_pp = acc
    S_pad = -(-max(S_pp, 1) // KSPAN) * KSPAN

    pr = rank[row]
    blk_e = pr // P
    p_e = (pr % P).astype(np.int64)
    g_e = blk_e // N_CORES
    j_e = blk_e % N_CORES
    core_e = np.where(g_e % 2 == 0, j_e, N_CORES - 1 - j_e)
    if E > 0:
        o = np.argsort(pr, kind="stable")
        pr_s = pr[o]
        first = np.r_[True, pr_s[1:] != pr_s[:-1]]
        run_start = np.maximum.accumulate(np.where(first, np.arange(E), 0))
        k_s = np.arange(E) - run_start
        k_e = np.empty(E, np.int64)
        k_e[o] = k_s
    else:
        k_e = np.zeros(0, np.int64)
    off_e = slot_base[g_e] + k_e

    col_slots = np.zeros((N_CORES, P, S_pad), np.int32)
    w_slots = np.zeros((N_CORES, P, S_pad), np.float32)
    wdi = (w.astype(np.float64) * deg_inv[row].astype(np.float64)).astype(np.float32)
    col_slots[core_e, p_e, off_e] = col
    w_slots[core_e, p_e, off_e] = wdi

    col_used = np.zeros((N_CORES, S_pad), bool)
    col_used[core_e, off_e] = True
    used_any = col_used.any(axis=0)

    return dict(order=order, K_sched=K_sched, slot_base=slot_base, S_pp=S_pp,
                S_pad=S_pad, G=G, col_slots=col_slots, w_slots=w_slots,
                used_any=used_any, N_pad=N_pad)


def _build_kernel(sched, N, gather_bufs=8):
    import concourse.bass as bass
    import concourse.bacc as bacc
    import concourse.tile as tile
    import concourse.mybir as mybir

    K_sched = sched["K_sched"]
    slot_base = sched["slot_base"]
    S_pad = sched["S_pad"]
    G = sched["G"]
    used_any = sched["used_any"]
    n_spans = S_pad // KSPAN

    nc = bacc.Bacc("TRN2", target_bir_lowering=False, debug=False,
                   num_devices=N_CORES)

    x = nc.dram_tensor("x", [N, F], mybir.dt.float32, kind="ExternalInput")
    cols = nc.dram_tensor("cols", [P, S_pad], mybir.dt.int32, kind="ExternalInput")
    ws = nc.dram_tensor("ws", [P, S_pad], mybir.dt.float32, kind="ExternalInput")
    out = nc.dram_tensor("out", [G * P, F], mybir.dt.float32, kind="ExternalOutput")

    with tile.TileContext(nc) as tc:
        with (
            tc.tile_pool(name="cp", bufs=1) as cp,
            tc.tile_pool(name="wp", bufs=1) as wp,
            tc.tile_pool(name="gp", bufs=gather_bufs) as gp,
            tc.tile_pool(name="rp", bufs=4) as rp,
        ):
            extents = []
            for s in range(n_spans):
                u = used_any[s * KSPAN:(s + 1) * KSPAN]
                ext = int(np.max(np.nonzero(u)[0]) + 1) if u.any() else 0
                assert u[:ext].all(), "span used columns not a prefix"
                extents.append(ext)

            # all offsets and weights resident up front (tiny: ~14 KB/partition)
            col_all = cp.tile([P, S_pad], mybir.dt.int32)
            nc.sync.dma_start(out=col_all[:], in_=cols[:])
            w_all = wp.tile([P, S_pad], mybir.dt.float32)
            nc.sync.dma_start(out=w_all[:], in_=ws[:])

            span_tiles = {}
            for s in range(n_spans):
                ext = extents[s]
                if ext == 0:
                    span_tiles[s] = None
                    continue
                g_t = gp.tile([P, KSPAN, F], mybir.dt.float32, tag="g")
                for j in range(ext):
                    jj = s * KSPAN + j
                    nc.gpsimd.indirect_dma_start(
                        out=g_t[:, j, :], out_offset=None, in_=x[:],
                        in_offset=bass.IndirectOffsetOnAxis(
                            ap=col_all[:, jj:jj + 1], axis=0))
                nc.vector.tensor_tensor(
                    out=g_t[:, 0:ext, :], in0=g_t[:, 0:ext, :],
                    in1=w_all[:, s * KSPAN:s * KSPAN + ext].to_broadcast(
                        [P, ext, F]),
                    op=mybir.AluOpType.mult)
                span_tiles[s] = g_t

            g = 0
            while g < G:
                k = int(K_sched[g])
                if k == 0:
                    ge = g
                    while ge < G and int(K_sched[ge]) == 0:
                        ge += 1
                    rz = rp.tile([P, F], mybir.dt.float32, tag="r")
                    nc.vector.memset(rz[:], 0.0)
                    for gg in range(g, ge):
                        nc.sync.dma_start(out=out[gg * P:(gg + 1) * P, :], in_=rz[:])
                    g = ge
                    continue
                s = int(slot_base[g]) // KSPAN
                ge = g + 1
                while (ge < G and int(K_sched[ge]) == k
                       and int(slot_base[ge]) == int(slot_base[ge - 1]) + k
                       and int(slot_base[ge]) // KSPAN == s):
                    ge += 1
                nrun = ge - g
                j0 = int(slot_base[g]) - s * KSPAN
                g_t = span_tiles[s]
                src = g_t[:, j0:j0 + nrun * k, :].rearrange(
                    "p (r k) f -> p r f k", k=k)
                r_t = rp.tile([P, nrun * F], mybir.dt.float32, tag="r")
                nc.vector.tensor_reduce(out=r_t[:], in_=src,
                                        axis=mybir.AxisListType.X,
                                        op=mybir.AluOpType.add)
                for i, gg in enumerate(range(g, ge)):
                    nc.sync.dma_start(out=out[gg * P:(gg + 1) * P, :],
                                      in_=r_t[:, i * F:(i + 1) * F])
                g = ge

    nc.compile()
    return nc


def _unshard(sched, core_outs, N):
    G = sched["G"]
    order = sched["order"]
    out = np.zeros((N, F), np.float32)
    g_idx = np.arange(G)
    for c in range(N_CORES):
        j = np.where(g_idx % 2 == 0, c, N_CORES - 1 - c)
        blk = g_idx * N_CORES + j
        ranks = (blk[:, None] * P + np.arange(P)).ravel()
        rows = order[ranks]
        mask = rows < N
        out[rows[mask]] = core_outs[c][mask]
    return out


def kernel(x, edge_index, edge_weight, deg_inv):
    global LAST_EXEC_TIME_NS
    import os
    from concourse.bass_utils import run_bass_kernel_spmd

    x = np.ascontiguousarray(np.asarray(x, dtype=np.float32))
    edge_index = np.asarray(edge_index, dtype=np.int32)
    edge_weight = np.asarray(edge_weight, dtype=np.float32)
    deg_inv = np.asarray(deg_inv, dtype=np.float32)
    N = x.shape[0]

    sched = _build_schedule(edge_index[0], edge_index[1], edge_weight, deg_inv, N)
    nc = _build_kernel(sched, N)
    in_maps = [{"x": x, "cols": sched["col_slots"][c], "ws": sched["w_slots"][c]}
               for c in range(N_CORES)]

    trace = bool(int(os.environ.get("KERNEL_TRACE", "0")))
    res = run_bass_kernel_spmd(nc, in_maps, core_ids=list(range(N_CORES)),
                               trace=trace)
    if trace:
        LAST_EXEC_TIME_NS = res.exec_time_ns
    return _unshard(sched, [r["out"] for r in res.results], N)
